# revision 12
# baseline (speedup 1.0000x reference)
"""Trainium2 Bass kernel for the nn_LSTMCell problem.

Strategy: data-parallel over the batch dim (4096 -> 8 cores x 512), weights
replicated. All on-chip compute happens in "transposed" orientation
(hidden on PSUM partitions, batch on the free dim) so every matmul operand
can be DMA'd in its natural, contiguous layout:

    gate.T[h, b] = sum_k W.T[k, h] * act.T[k, b]

Matmuls run in fp8e4 (DoubleRow, 2 k-tiles per instruction, ~1.8x bf16
instruction throughput) except the g-gate (tanh path: no sigmoid
attenuation, dominant error contributor), which stays bf16. All weights
(both dtypes) are pre-scaled x256 on the host so the fp8 ones sit in
e4m3's normal range (raw |W|<=0.023 would quantize as subnormals with
~20% relative error); the 1/256 is folded into the scalar-engine
activation instruction: out = func(psum/256 + bias). PSUM accumulation is
fp32; all elementwise math and outputs are fp32.

Per core:
  phase 1: per h-tile: i/f gate fp8 matmuls + g gate bf16 matmuls,
           sigmoid/tanh, c1 = f*c0 + i*tanh(g) (fp32, kept in SBUF +
           DMA'd out), c1 cast to fp8 (o-gate matmul operand).
  phase 2: per h-tile: o gate fp8 matmuls (incl. W_co @ c1.T),
           o = sigmoid(...), h1 = o * tanh(c1), DMA out.
"""

import numpy as np
import ml_dtypes
from contextlib import ExitStack

BF = ml_dtypes.bfloat16
F8 = ml_dtypes.float8_e4m3   # TRN FP8_EXP4 (max +-240)
W_SCALE = 256.0              # weights pre-scaled into e4m3 normal range

N_CORES = 8
P = 128          # partition dim / k-tile size / m-tile size
BATCH = 4096
IN_DIM = 2048
HID = 2048
B = BATCH // N_CORES          # 512, batch per core = matmul free dim
KI = IN_DIM // P              # 16, k-tiles for x contraction
KH = HID // P                 # 16, k-tiles for h/c contraction
MT = HID // P                 # 16, output h-tiles

W_NAMES = ["ii", "hi", "if_", "hf", "cf", "ic", "hc", "io", "ho", "co"]
X_NAMES = ("ii", "if_", "ic", "io")   # weights contracting over x
# matmuls run in fp8 DoubleRow except the g-gate (tanh path)
FP8_SET = frozenset(W_NAMES) - {"ic", "hc"}


def _build(p, ki, kh, mt, b, fp8_set):
    import concourse.tile as tile
    from concourse import bacc, mybir

    bf16, f32, f8 = mybir.dt.bfloat16, mybir.dt.float32, mybir.dt.float8e4
    Sig = mybir.ActivationFunctionType.Sigmoid
    Tanh = mybir.ActivationFunctionType.Tanh
    DR = mybir.MatmulPerfMode.DoubleRow
    inv_s = 1.0 / W_SCALE

    nc = bacc.Bacc(
        "TRN2",
        target_bir_lowering=False,
        debug=False,
        num_devices=N_CORES,
    )

    def wdt(n):
        return f8 if n in fp8_set else bf16

    # which activation dtypes are needed
    need8 = {a: any(n in fp8_set for n in ns) for a, ns in
             (("x", X_NAMES), ("h", ("hi", "hf", "hc", "ho")), ("c", ("cf",)))}
    needb = {a: any(n not in fp8_set for n in ns) for a, ns in
             (("x", X_NAMES), ("h", ("hi", "hf", "hc", "ho")), ("c", ("cf",)))}

    dram_act = {}
    for a in ("x", "h", "c"):
        if need8[a]:
            dram_act[a + "8"] = nc.dram_tensor(
                a + "T8", [p, ki if a == "x" else kh, b], f8, kind="ExternalInput").ap()
        if needb[a]:
            dram_act[a + "b"] = nc.dram_tensor(
                a + "Tb", [p, ki if a == "x" else kh, b], bf16, kind="ExternalInput").ap()
    c0T = nc.dram_tensor("c0T", [p, mt, b], f32, kind="ExternalInput").ap()
    bias = nc.dram_tensor("bias", [p, mt, 4], f32, kind="ExternalInput").ap()
    w = {
        n: nc.dram_tensor(
            f"w_{n}", [mt, p, (ki if n in X_NAMES else kh), p],
            wdt(n), kind="ExternalInput",
        ).ap()
        for n in W_NAMES
    }
    ogT = nc.dram_tensor("ogT", [p, mt, b], f32, kind="ExternalOutput").ap()
    h1T = nc.dram_tensor("h1T", [p, mt, b], f32, kind="ExternalOutput").ap()
    c1T = nc.dram_tensor("c1T", [p, mt, b], f32, kind="ExternalOutput").ap()

    with tile.TileContext(nc) as tc, ExitStack() as ctx:
        acts = ctx.enter_context(tc.tile_pool(name="acts", bufs=1))
        wpool = ctx.enter_context(tc.tile_pool(name="w", bufs=2))
        cpool = ctx.enter_context(tc.tile_pool(name="c0", bufs=2))
        tpool = ctx.enter_context(tc.tile_pool(name="temps", bufs=2))
        ppool = ctx.enter_context(tc.tile_pool(name="psum", bufs=8, space="PSUM"))

        # resident activations. Loads are chunked and issued in first-use
        # order across four DMA issue queues so the PE's software-pipelined
        # prologue (x-side matmuls of m=0/1) streams while h/c still load.
        CH = 4  # k-tiles per DMA chunk
        act_sb = {}
        for key, src in dram_act.items():
            nk = src.shape[1]
            act_sb[key] = acts.tile([p, nk, b], f8 if key.endswith("8") else bf16,
                                    tag=key, name=key + "_sb")

        def load_act(key, eng, lo=0, hi=None, ch=CH):
            src = dram_act[key]
            dst = act_sb[key]
            hi = src.shape[1] if hi is None else hi
            for c in range(lo, hi, ch):
                eng.dma_start(dst[:, c:c + ch, :], src[:, c:c + ch, :])

        bias_sb = acts.tile([p, mt, 4], f32, tag="bias")
        c1f_sb = acts.tile([p, mt, b], f32, tag="c1f")    # new cell state, fp32
        # copy of c1 in the o-gate matmul operand dtype
        c1m_dt = f8 if "co" in fp8_set else bf16
        c1m_sb = acts.tile([p, mt, b], c1m_dt, tag="c1m")

        def load_w(name, tag, m, chunks=1, eng=None):
            nk = w[name].shape[2]
            t = wpool.tile([p, nk, p], wdt(name), tag=tag)
            step = max(1, nk // chunks)
            for c in range(0, nk, step):
                (eng or nc.sync).dma_start(t[:, c:c + step], w[name][m, :, c:c + step])
            return t

        def accum(ps, name, w_t, act_key, first, last):
            fp8 = name in fp8_set
            if isinstance(act_key, str):
                a = act_sb[act_key + ("8" if fp8 else "b")]
            else:
                a = act_key
            nk = w_t.shape[1]
            if fp8:
                for t in range(0, nk, 2):
                    nc.tensor.matmul(
                        ps[:], lhsT=w_t[:, t:t + 2, :], rhs=a[:, t:t + 2, :],
                        start=(first and t == 0), stop=(last and t == nk - 2),
                        perf_mode=DR,
                    )
            else:
                for t in range(nk):
                    nc.tensor.matmul(
                        ps[:], lhsT=w_t[:, t], rhs=a[:, t],
                        start=(first and t == 0), stop=(last and t == nk - 1),
                    )

        # ---- phase 1: i/f/g gates + new cell state ----
        # Software-pipelined: the x-side accumulations of m-tiles 0/1 run as
        # a prologue (only x + x-weights needed on chip), and each iteration
        # m finishes gates for m while queueing the x-side work of m+2 at
        # the end. This keeps the PE dense from t~1us so the initial h/c
        # activation loads hide behind real matmul work.
        xw = {}       # m -> (w_ii, w_if, w_ic) tiles
        pend = {}     # m -> {"i"/"f"/"g": psum tile}
        p2w = {}      # m -> (w_io, w_ho, w_co) tiles

        def load_xw(m, eng=None, chunks=1):
            xw[m] = (load_w("ii", "wxi", m, chunks, eng),
                     load_w("if_", "wxf", m, chunks, eng),
                     load_w("ic", "wxg", m, chunks, eng))

        def load_ow(m, eng=None):
            p2w[m] = (load_w("io", "po", m, eng=eng),
                      load_w("ho", "ph", m, eng=eng),
                      load_w("co", "pc", m, eng=eng))

        def x_parts_if(m):
            ps = pend.setdefault(m, {})
            ps["i"] = ppool.tile([p, b], f32, tag="ps", name=f"ps_i{m}")
            accum(ps["i"], "ii", xw[m][0], "x", True, False)
            ps["f"] = ppool.tile([p, b], f32, tag="ps", name=f"ps_f{m}")
            accum(ps["f"], "if_", xw[m][1], "x", True, False)

        def x_parts_g(m):
            ps = pend[m]
            ps["g"] = ppool.tile([p, b], f32, tag="ps", name=f"ps_g{m}")
            accum(ps["g"], "ic", xw[m][2], "x", True, False)
            del xw[m]

        # prologue: need-ordered initial loads + x-side work of m=0/1
        load_act("x8", nc.gpsimd, ch=2)
        load_xw(0, eng=nc.scalar, chunks=2)
        load_xw(1, eng=nc.scalar)
        if "xb" in act_sb:
            load_act("xb", nc.sync)
        hw_tags = ("hi", "w1"), ("hf", "w3"), ("hc", "w6"), ("cf", "w4")
        w_h = {}
        for (n, tag), eng in zip(hw_tags, (nc.scalar, nc.scalar, nc.sync, nc.sync)):
            w_h[n, 0] = load_w(n, tag, 0, eng=eng)
        load_act("h8", nc.gpsimd)
        if "hb" in act_sb:
            load_act("hb", nc.sync)
        load_act("c8" if "c8" in act_sb else "cb", nc.gpsimd)
        nc.gpsimd.dma_start(bias_sb[:], bias[:])
        for (n, tag) in hw_tags:
            w_h[n, 1] = load_w(n, tag, 1, eng=nc.sync)
        x_parts_if(0)
        x_parts_if(1)
        x_parts_g(0)
        x_parts_g(1)

        for m in range(mt):
            if m + 2 < mt:
                load_xw(m + 2)
            if m + 1 < mt and (m + 1) > 1:
                for n, tag in hw_tags:
                    w_h[n, m + 1] = load_w(n, tag, m + 1)

            ps = pend.pop(m)
            ps_i, ps_f, ps_g = ps["i"], ps["f"], ps["g"]
            accum(ps_i, "hi", w_h.pop(("hi", m)), "h", False, True)
            accum(ps_f, "hf", w_h.pop(("hf", m)), "h", False, False)
            accum(ps_g, "hc", w_h.pop(("hc", m)), "h", False, True)
            accum(ps_f, "cf", w_h.pop(("cf", m)), "c", False, True)

            i_act = tpool.tile([p, b], f32, tag="i_act")
            nc.scalar.activation(i_act[:], ps_i[:], Sig,
                                 bias=bias_sb[:, m, 0:1], scale=inv_s)
            f_act = tpool.tile([p, b], f32, tag="f_act")
            nc.scalar.activation(f_act[:], ps_f[:], Sig,
                                 bias=bias_sb[:, m, 1:2], scale=inv_s)
            g_act = tpool.tile([p, b], f32, tag="g_act")
            nc.scalar.activation(g_act[:], ps_g[:], Tanh,
                                 bias=bias_sb[:, m, 2:3], scale=inv_s)

            c0_t = cpool.tile([p, b], f32, tag="c0")
            nc.gpsimd.dma_start(c0_t[:], c0T[:, m, :])

            t1 = tpool.tile([p, b], f32, tag="t1")
            nc.vector.tensor_mul(t1[:], f_act[:], c0_t[:])
            nc.vector.tensor_mul(i_act[:], i_act[:], g_act[:])
            c1_m = c1f_sb[:, m, :]
            nc.vector.tensor_add(c1_m, t1[:], i_act[:])
            nc.vector.tensor_copy(out=c1m_sb[:, m, :], in_=c1_m)
            nc.sync.dma_start(c1T[:, m, :], c1_m)

            if m == mt - 2:
                # prefetch phase-2 m=0 weights on the lightly-loaded gpsimd
                # queue so phase 2's first matmuls don't wait on sync.
                load_ow(0, eng=nc.gpsimd)
            if m + 2 < mt:
                x_parts_if(m + 2)
                x_parts_g(m + 2)

        # ---- phase 2: o gate + h1 ----
        for m in range(mt):
            if m + 1 < mt:
                load_ow(m + 1)
            w_io, w_ho, w_co = p2w.pop(m)

            ps_o = ppool.tile([p, b], f32, tag="ps")
            accum(ps_o, "io", w_io, "x", True, False)
            accum(ps_o, "ho", w_ho, "h", False, False)
            accum(ps_o, "co", w_co, c1m_sb, False, True)

            o_act = tpool.tile([p, b], f32, tag="o_act")
            nc.scalar.activation(o_act[:], ps_o[:], Sig,
                                 bias=bias_sb[:, m, 3:4], scale=inv_s)
            tc1 = tpool.tile([p, b], f32, tag="tc1")
            nc.scalar.activation(tc1[:], c1f_sb[:, m, :], Tanh)
            h1_t = tpool.tile([p, b], f32, tag="h1")
            nc.vector.tensor_mul(h1_t[:], o_act[:], tc1[:])

            # outputs spread across three queues to avoid a drain backlog
            # at the end of the kernel.
            nc.scalar.dma_start(ogT[:, m, :], o_act[:])
            nc.gpsimd.dma_start(h1T[:, m, :b // 2], h1_t[:, :b // 2])
            nc.sync.dma_start(h1T[:, m, b // 2:], h1_t[:, b // 2:])

    nc.compile()
    return nc


_NC = None
_NC_KEY = None


def _get_nc():
    global _NC, _NC_KEY
    key = frozenset(FP8_SET)
    if _NC is None or _NC_KEY != key:
        _NC = _build(P, KI, KH, MT, B, key)
        _NC_KEY = key
    return _NC


# ---------------- host-side packing ----------------

def _pack_actT(a, dtype):
    """(b, d) -> (128, d//128, b) with [ki, ko, b] = a[b, ko*128+ki]."""
    b, d = a.shape
    return np.ascontiguousarray(
        a.T.reshape(d // P, P, b).transpose(1, 0, 2)
    ).astype(dtype, copy=False)


def _pack_w(W, dtype):
    """(H, K) -> (H//128, 128, K//128, 128) with [mt, ki, ko, m] = s*W[mt*128+m, ko*128+ki]."""
    H, K = W.shape
    return np.ascontiguousarray(
        (W.reshape(H // P, P, K // P, P) * W_SCALE)
        .transpose(0, 3, 2, 1).astype(dtype)
    )


def _unpack_out(o):
    """(128, mt, b) [p, m, b] -> (b, mt*128)."""
    p, m, b = o.shape
    return np.ascontiguousarray(o.transpose(2, 1, 0).reshape(b, m * p))


def kernel(x, h0, c0,
           W_ii, b_ii, W_hi, b_hi, W_if_, b_if_, W_hf, b_hf, W_cf, b_cf,
           W_ic, b_ic, W_hc, b_hc, W_io, b_io, W_ho, b_ho, W_co, b_co,
           _trace=False):
    from concourse.bass_utils import run_bass_kernel_spmd

    nc = _get_nc()

    x = np.asarray(x, dtype=np.float32)
    h0 = np.asarray(h0, dtype=np.float32)
    c0 = np.asarray(c0, dtype=np.float32)
    Ws = dict(zip(W_NAMES, [W_ii, W_hi, W_if_, W_hf, W_cf,
                            W_ic, W_hc, W_io, W_ho, W_co]))
    Ws = {n: np.asarray(a, dtype=np.float32) for n, a in Ws.items()}
    (b_ii, b_hi, b_if_, b_hf, b_cf, b_ic, b_hc, b_io, b_ho, b_co) = [
        np.asarray(a, dtype=np.float32)
        for a in (b_ii, b_hi, b_if_, b_hf, b_cf, b_ic, b_hc, b_io, b_ho, b_co)
    ]

    # combined per-gate biases, packed [p, mt, gate]
    bias = np.stack(
        [
            (b_ii + b_hi).reshape(MT, P).T,
            (b_if_ + b_hf + b_cf).reshape(MT, P).T,
            (b_ic + b_hc).reshape(MT, P).T,
            (b_io + b_ho + b_co).reshape(MT, P).T,
        ],
        axis=2,
    ).astype(np.float32)
    w_packed = {
        f"w_{n}": _pack_w(W, F8 if n in FP8_SET else BF)
        for n, W in Ws.items()
    }

    need8 = {"x": any(n in FP8_SET for n in X_NAMES),
             "h": any(n in FP8_SET for n in ("hi", "hf", "hc", "ho")),
             "c": "cf" in FP8_SET}
    needb = {"x": any(n not in FP8_SET for n in X_NAMES),
             "h": any(n not in FP8_SET for n in ("hi", "hf", "hc", "ho")),
             "c": "cf" not in FP8_SET}

    in_maps = []
    for core in range(N_CORES):
        s = slice(core * B, (core + 1) * B)
        m = {"c0T": _pack_actT(c0[s], np.float32), "bias": bias}
        for a, full in (("x", x), ("h", h0), ("c", c0)):
            if need8[a]:
                m[a + "T8"] = _pack_actT(full[s], F8)
            if needb[a]:
                m[a + "Tb"] = _pack_actT(full[s], BF)
        m.update(w_packed)
        in_maps.append(m)

    res = run_bass_kernel_spmd(nc, in_maps, list(range(N_CORES)), trace=_trace)

    o_g = np.empty((BATCH, HID), np.float32)
    h1 = np.empty((BATCH, HID), np.float32)
    c1 = np.empty((BATCH, HID), np.float32)
    for core in range(N_CORES):
        s = slice(core * B, (core + 1) * B)
        o_g[s] = _unpack_out(res.results[core]["ogT"])
        h1[s] = _unpack_out(res.results[core]["h1T"])
        c1[s] = _unpack_out(res.results[core]["c1T"])
    out = (o_g, h1, c1)
    if _trace:
        return out, res
    return out


# revision 16
# speedup vs baseline: 1.0071x; 1.0071x over previous
"""Trainium2 Bass kernel for the nn_LSTMCell problem.

Strategy: data-parallel over the batch dim (4096 -> 8 cores x 512), weights
replicated. All on-chip compute happens in "transposed" orientation
(hidden on PSUM partitions, batch on the free dim) so every matmul operand
can be DMA'd in its natural, contiguous layout:

    gate.T[h, b] = sum_k W.T[k, h] * act.T[k, b]

Matmuls run in fp8e4 (DoubleRow, 2 k-tiles per instruction, ~1.8x bf16
instruction throughput) except the g-gate (tanh path: no sigmoid
attenuation, dominant error contributor), which stays bf16. All weights
(both dtypes) are pre-scaled x256 on the host so the fp8 ones sit in
e4m3's normal range (raw |W|<=0.023 would quantize as subnormals with
~20% relative error); the 1/256 is folded into the scalar-engine
activation instruction: out = func(psum/256 + bias). PSUM accumulation is
fp32; all elementwise math and outputs are fp32.

Per core:
  phase 1: per h-tile: i/f gate fp8 matmuls + g gate bf16 matmuls,
           sigmoid/tanh, c1 = f*c0 + i*tanh(g) (fp32, kept in SBUF +
           DMA'd out), c1 cast to fp8 (o-gate matmul operand).
  phase 2: per h-tile: o gate fp8 matmuls (incl. W_co @ c1.T),
           o = sigmoid(...), h1 = o * tanh(c1), DMA out.
"""

import numpy as np
import ml_dtypes
from contextlib import ExitStack

BF = ml_dtypes.bfloat16
F8 = ml_dtypes.float8_e4m3   # TRN FP8_EXP4 (max +-240)
W_SCALE = 256.0              # weights pre-scaled into e4m3 normal range

N_CORES = 8
P = 128          # partition dim / k-tile size / m-tile size
BATCH = 4096
IN_DIM = 2048
HID = 2048
B = BATCH // N_CORES          # 512, batch per core = matmul free dim
KI = IN_DIM // P              # 16, k-tiles for x contraction
KH = HID // P                 # 16, k-tiles for h/c contraction
MT = HID // P                 # 16, output h-tiles

W_NAMES = ["ii", "hi", "if_", "hf", "cf", "ic", "hc", "io", "ho", "co"]
X_NAMES = ("ii", "if_", "ic", "io")   # weights contracting over x
# matmuls run in fp8 DoubleRow except the g-gate (tanh path)
FP8_SET = frozenset(W_NAMES) - {"ic", "hc"}


def _build(p, ki, kh, mt, b, fp8_set):
    import concourse.tile as tile
    from concourse import bacc, mybir

    bf16, f32, f8 = mybir.dt.bfloat16, mybir.dt.float32, mybir.dt.float8e4
    Sig = mybir.ActivationFunctionType.Sigmoid
    Tanh = mybir.ActivationFunctionType.Tanh
    DR = mybir.MatmulPerfMode.DoubleRow
    inv_s = 1.0 / W_SCALE

    nc = bacc.Bacc(
        "TRN2",
        target_bir_lowering=False,
        debug=False,
        num_devices=N_CORES,
    )

    def wdt(n):
        return f8 if n in fp8_set else bf16

    # which activation dtypes are needed
    need8 = {a: any(n in fp8_set for n in ns) for a, ns in
             (("x", X_NAMES), ("h", ("hi", "hf", "hc", "ho")), ("c", ("cf",)))}
    needb = {a: any(n not in fp8_set for n in ns) for a, ns in
             (("x", X_NAMES), ("h", ("hi", "hf", "hc", "ho")), ("c", ("cf",)))}

    dram_act = {}
    for a in ("x", "h", "c"):
        if need8[a]:
            dram_act[a + "8"] = nc.dram_tensor(
                a + "T8", [p, ki if a == "x" else kh, b], f8, kind="ExternalInput").ap()
        if needb[a]:
            dram_act[a + "b"] = nc.dram_tensor(
                a + "Tb", [p, ki if a == "x" else kh, b], bf16, kind="ExternalInput").ap()
    c0T = nc.dram_tensor("c0T", [p, mt, b], f32, kind="ExternalInput").ap()
    bias = nc.dram_tensor("bias", [p, mt, 4], f32, kind="ExternalInput").ap()
    w = {
        n: nc.dram_tensor(
            f"w_{n}", [mt, p, (ki if n in X_NAMES else kh), p],
            wdt(n), kind="ExternalInput",
        ).ap()
        for n in W_NAMES
    }
    ogT = nc.dram_tensor("ogT", [p, mt, b], f32, kind="ExternalOutput").ap()
    h1T = nc.dram_tensor("h1T", [p, mt, b], f32, kind="ExternalOutput").ap()
    c1T = nc.dram_tensor("c1T", [p, mt, b], f32, kind="ExternalOutput").ap()

    with tile.TileContext(nc) as tc, ExitStack() as ctx:
        acts = ctx.enter_context(tc.tile_pool(name="acts", bufs=1))
        wpool = ctx.enter_context(tc.tile_pool(name="w", bufs=2))
        cpool = ctx.enter_context(tc.tile_pool(name="c0", bufs=2))
        tpool = ctx.enter_context(tc.tile_pool(name="temps", bufs=2))
        ppool = ctx.enter_context(tc.tile_pool(name="psum", bufs=8, space="PSUM"))

        # resident activations. Loads are chunked and issued in first-use
        # order across four DMA issue queues so the PE's software-pipelined
        # prologue (x-side matmuls of m=0/1) streams while h/c still load.
        CH = 4  # k-tiles per DMA chunk
        act_sb = {}
        for key, src in dram_act.items():
            nk = src.shape[1]
            act_sb[key] = acts.tile([p, nk, b], f8 if key.endswith("8") else bf16,
                                    tag=key, name=key + "_sb")

        def load_act(key, eng, lo=0, hi=None, ch=CH):
            src = dram_act[key]
            dst = act_sb[key]
            hi = src.shape[1] if hi is None else hi
            for c in range(lo, hi, ch):
                eng.dma_start(dst[:, c:c + ch, :], src[:, c:c + ch, :])

        bias_sb = acts.tile([p, mt, 4], f32, tag="bias")
        c1f_sb = acts.tile([p, mt, b], f32, tag="c1f")    # new cell state, fp32
        # copy of c1 in the o-gate matmul operand dtype
        c1m_dt = f8 if "co" in fp8_set else bf16
        c1m_sb = acts.tile([p, mt, b], c1m_dt, tag="c1m")

        def load_w(name, tag, m, chunks=1, eng=None):
            nk = w[name].shape[2]
            t = wpool.tile([p, nk, p], wdt(name), tag=tag)
            step = max(1, nk // chunks)
            for c in range(0, nk, step):
                (eng or nc.sync).dma_start(t[:, c:c + step], w[name][m, :, c:c + step])
            return t

        def accum(ps, name, w_t, act_key, first, last):
            fp8 = name in fp8_set
            if isinstance(act_key, str):
                a = act_sb[act_key + ("8" if fp8 else "b")]
            else:
                a = act_key
            nk = w_t.shape[1]
            if fp8:
                for t in range(0, nk, 2):
                    nc.tensor.matmul(
                        ps[:], lhsT=w_t[:, t:t + 2, :], rhs=a[:, t:t + 2, :],
                        start=(first and t == 0), stop=(last and t == nk - 2),
                        perf_mode=DR,
                    )
            else:
                for t in range(nk):
                    nc.tensor.matmul(
                        ps[:], lhsT=w_t[:, t], rhs=a[:, t],
                        start=(first and t == 0), stop=(last and t == nk - 1),
                    )

        # ---- phase 1: i/f/g gates + new cell state ----
        # Software-pipelined: the x-side accumulations of m-tiles 0/1 run as
        # a prologue (only x + x-weights needed on chip), and each iteration
        # m finishes gates for m while queueing the x-side work of m+2 at
        # the end. This keeps the PE dense from t~1us so the initial h/c
        # activation loads hide behind real matmul work.
        xw = {}       # m -> (w_ii, w_if) tiles
        wg = {}       # m -> w_ic tile
        pend = {}     # m -> {"i"/"f": psum tile}
        p2w = {}      # m -> (w_io, w_ho, w_co) tiles

        def load_xw(m, eng=None, chunks=1):
            xw[m] = (load_w("ii", "wxi", m, chunks, eng),
                     load_w("if_", "wxf", m, chunks, eng))

        def load_ow(m, eng=None):
            p2w[m] = (load_w("io", "po", m, eng=eng),
                      load_w("ho", "ph", m, eng=eng),
                      load_w("co", "pc", m, eng=eng))

        def x_parts_if(m):
            ps = pend.setdefault(m, {})
            ps["i"] = ppool.tile([p, b], f32, tag="ps", name=f"ps_i{m}")
            accum(ps["i"], "ii", xw[m][0], "x", True, False)
            ps["f"] = ppool.tile([p, b], f32, tag="ps", name=f"ps_f{m}")
            accum(ps["f"], "if_", xw[m][1], "x", True, False)
            del xw[m]

        # prologue. The DMA path takes ~10us to move its first bytes, so
        # the PE's earliest work is arranged to need as few bytes as
        # possible: i/f x-side accums of m=0/1 (x8 + four small fp8 slabs,
        # ~1.75MB). The bf16 g-gate work (xb + w_ic, 2.5MB) is deferred
        # into each iteration. Tiny bias pokes spin up all three queues.
        nc.gpsimd.dma_start(bias_sb[:, 0:8], bias[:, 0:8])
        nc.sync.dma_start(bias_sb[:, 8:12], bias[:, 8:12])
        nc.scalar.dma_start(bias_sb[:, 12:16], bias[:, 12:16])
        load_act("x8", nc.gpsimd)
        load_xw(0, eng=nc.scalar, chunks=2)
        load_xw(1, eng=nc.scalar)
        hw_tags = ("hi", "w1"), ("hf", "w3"), ("hc", "w6"), ("cf", "w4")
        w_h = {}
        load_act("h8", nc.sync)
        w_h["hi", 0] = load_w("hi", "w1", 0, chunks=2, eng=nc.scalar)
        w_h["hf", 0] = load_w("hf", "w3", 0, chunks=2, eng=nc.scalar)
        w_h["cf", 0] = load_w("cf", "w4", 0, eng=nc.sync)
        if "xb" in act_sb:
            load_act("xb", nc.sync)
        wg[0] = load_w("ic", "wxg", 0, eng=nc.scalar)
        w_h["hc", 0] = load_w("hc", "w6", 0, eng=nc.sync)
        if "hb" in act_sb:
            load_act("hb", nc.gpsimd)
        load_act("c8" if "c8" in act_sb else "cb", nc.gpsimd)
        for (n, tag) in hw_tags:
            w_h[n, 1] = load_w(n, tag, 1, eng=nc.sync)
        x_parts_if(0)
        x_parts_if(1)

        for m in range(mt):
            if m + 2 < mt:
                load_xw(m + 2)
            if m + 1 < mt:
                wg[m + 1] = load_w("ic", "wxg", m + 1, eng=nc.scalar)
                if m + 1 > 1:
                    for n, tag in hw_tags:
                        w_h[n, m + 1] = load_w(n, tag, m + 1)

            ps = pend.pop(m)
            ps_i, ps_f = ps["i"], ps["f"]
            accum(ps_i, "hi", w_h.pop(("hi", m)), "h", False, True)
            i_act = tpool.tile([p, b], f32, tag="i_act")
            nc.scalar.activation(i_act[:], ps_i[:], Sig,
                                 bias=bias_sb[:, m, 0:1], scale=inv_s)
            accum(ps_f, "hf", w_h.pop(("hf", m)), "h", False, False)
            accum(ps_f, "cf", w_h.pop(("cf", m)), "c", False, True)
            f_act = tpool.tile([p, b], f32, tag="f_act")
            nc.scalar.activation(f_act[:], ps_f[:], Sig,
                                 bias=bias_sb[:, m, 1:2], scale=inv_s)
            ps_g = ppool.tile([p, b], f32, tag="ps", name=f"ps_g{m}")
            accum(ps_g, "ic", wg.pop(m), "x", True, False)
            accum(ps_g, "hc", w_h.pop(("hc", m)), "h", False, True)
            g_act = tpool.tile([p, b], f32, tag="g_act")
            nc.scalar.activation(g_act[:], ps_g[:], Tanh,
                                 bias=bias_sb[:, m, 2:3], scale=inv_s)

            c0_t = cpool.tile([p, b], f32, tag="c0")
            nc.gpsimd.dma_start(c0_t[:], c0T[:, m, :])

            t1 = tpool.tile([p, b], f32, tag="t1")
            nc.vector.tensor_mul(t1[:], f_act[:], c0_t[:])
            nc.vector.tensor_mul(i_act[:], i_act[:], g_act[:])
            c1_m = c1f_sb[:, m, :]
            nc.vector.tensor_add(c1_m, t1[:], i_act[:])
            nc.vector.tensor_copy(out=c1m_sb[:, m, :], in_=c1_m)
            nc.sync.dma_start(c1T[:, m, :], c1_m)

            if m == mt - 2:
                # prefetch phase-2 m=0 weights on the lightly-loaded gpsimd
                # queue so phase 2's first matmuls don't wait on sync.
                load_ow(0, eng=nc.gpsimd)
            if m + 2 < mt:
                x_parts_if(m + 2)

        # ---- phase 2: o gate + h1 ----
        for m in range(mt):
            if m + 1 < mt:
                load_ow(m + 1)
            w_io, w_ho, w_co = p2w.pop(m)

            ps_o = ppool.tile([p, b], f32, tag="ps")
            accum(ps_o, "io", w_io, "x", True, False)
            accum(ps_o, "ho", w_ho, "h", False, False)
            accum(ps_o, "co", w_co, c1m_sb, False, True)

            o_act = tpool.tile([p, b], f32, tag="o_act")
            nc.scalar.activation(o_act[:], ps_o[:], Sig,
                                 bias=bias_sb[:, m, 3:4], scale=inv_s)
            tc1 = tpool.tile([p, b], f32, tag="tc1")
            nc.scalar.activation(tc1[:], c1f_sb[:, m, :], Tanh)
            h1_t = tpool.tile([p, b], f32, tag="h1")
            nc.vector.tensor_mul(h1_t[:], o_act[:], tc1[:])

            # outputs spread across three queues to avoid a drain backlog
            # at the end of the kernel.
            nc.scalar.dma_start(ogT[:, m, :], o_act[:])
            nc.gpsimd.dma_start(h1T[:, m, :b // 2], h1_t[:, :b // 2])
            nc.sync.dma_start(h1T[:, m, b // 2:], h1_t[:, b // 2:])

    nc.compile()
    return nc


_NC = None
_NC_KEY = None


def _get_nc():
    global _NC, _NC_KEY
    key = frozenset(FP8_SET)
    if _NC is None or _NC_KEY != key:
        _NC = _build(P, KI, KH, MT, B, key)
        _NC_KEY = key
    return _NC


# ---------------- host-side packing ----------------

def _pack_actT(a, dtype):
    """(b, d) -> (128, d//128, b) with [ki, ko, b] = a[b, ko*128+ki]."""
    b, d = a.shape
    return np.ascontiguousarray(
        a.T.reshape(d // P, P, b).transpose(1, 0, 2)
    ).astype(dtype, copy=False)


def _pack_w(W, dtype):
    """(H, K) -> (H//128, 128, K//128, 128) with [mt, ki, ko, m] = s*W[mt*128+m, ko*128+ki]."""
    H, K = W.shape
    return np.ascontiguousarray(
        (W.reshape(H // P, P, K // P, P) * W_SCALE)
        .transpose(0, 3, 2, 1).astype(dtype)
    )


def _unpack_out(o):
    """(128, mt, b) [p, m, b] -> (b, mt*128)."""
    p, m, b = o.shape
    return np.ascontiguousarray(o.transpose(2, 1, 0).reshape(b, m * p))


def kernel(x, h0, c0,
           W_ii, b_ii, W_hi, b_hi, W_if_, b_if_, W_hf, b_hf, W_cf, b_cf,
           W_ic, b_ic, W_hc, b_hc, W_io, b_io, W_ho, b_ho, W_co, b_co,
           _trace=False):
    from concourse.bass_utils import run_bass_kernel_spmd

    nc = _get_nc()

    x = np.asarray(x, dtype=np.float32)
    h0 = np.asarray(h0, dtype=np.float32)
    c0 = np.asarray(c0, dtype=np.float32)
    Ws = dict(zip(W_NAMES, [W_ii, W_hi, W_if_, W_hf, W_cf,
                            W_ic, W_hc, W_io, W_ho, W_co]))
    Ws = {n: np.asarray(a, dtype=np.float32) for n, a in Ws.items()}
    (b_ii, b_hi, b_if_, b_hf, b_cf, b_ic, b_hc, b_io, b_ho, b_co) = [
        np.asarray(a, dtype=np.float32)
        for a in (b_ii, b_hi, b_if_, b_hf, b_cf, b_ic, b_hc, b_io, b_ho, b_co)
    ]

    # combined per-gate biases, packed [p, mt, gate]
    bias = np.stack(
        [
            (b_ii + b_hi).reshape(MT, P).T,
            (b_if_ + b_hf + b_cf).reshape(MT, P).T,
            (b_ic + b_hc).reshape(MT, P).T,
            (b_io + b_ho + b_co).reshape(MT, P).T,
        ],
        axis=2,
    ).astype(np.float32)
    w_packed = {
        f"w_{n}": _pack_w(W, F8 if n in FP8_SET else BF)
        for n, W in Ws.items()
    }

    need8 = {"x": any(n in FP8_SET for n in X_NAMES),
             "h": any(n in FP8_SET for n in ("hi", "hf", "hc", "ho")),
             "c": "cf" in FP8_SET}
    needb = {"x": any(n not in FP8_SET for n in X_NAMES),
             "h": any(n not in FP8_SET for n in ("hi", "hf", "hc", "ho")),
             "c": "cf" not in FP8_SET}

    in_maps = []
    for core in range(N_CORES):
        s = slice(core * B, (core + 1) * B)
        m = {"c0T": _pack_actT(c0[s], np.float32), "bias": bias}
        for a, full in (("x", x), ("h", h0), ("c", c0)):
            if need8[a]:
                m[a + "T8"] = _pack_actT(full[s], F8)
            if needb[a]:
                m[a + "Tb"] = _pack_actT(full[s], BF)
        m.update(w_packed)
        in_maps.append(m)

    res = run_bass_kernel_spmd(nc, in_maps, list(range(N_CORES)), trace=_trace)

    o_g = np.empty((BATCH, HID), np.float32)
    h1 = np.empty((BATCH, HID), np.float32)
    c1 = np.empty((BATCH, HID), np.float32)
    for core in range(N_CORES):
        s = slice(core * B, (core + 1) * B)
        o_g[s] = _unpack_out(res.results[core]["ogT"])
        h1[s] = _unpack_out(res.results[core]["h1T"])
        c1[s] = _unpack_out(res.results[core]["c1T"])
    out = (o_g, h1, c1)
    if _trace:
        return out, res
    return out
